# revision 21
# baseline (speedup 1.0000x reference)
"""MoE gate (softmax + top-2 + load-balance loss) on 8 Trainium2 NeuronCores.

Data-parallel: token dim (16384) sharded 8 ways, gate weight replicated.
Per core: logits = x_shard @ W.T via PE (f32), softmax + top-2 via DVE/ACT,
partial per-expert prob sums returned per core; the scalar load-balance loss
is finished on host (tiny reduction).

x arrives [n, d] but the PE contracts along the partition axis, so each
core's shard is fed pre-transposed ([d, n], done on host) — natural-layout
DMA loads, no on-device transpose.

Matmul arrangement: the small gate weight W^T chunk [128, 64] is the
stationary operand (cheap reload), x streams as the moving operand with the
full 512-wide free dim, producing logits transposed [64 experts, 512 tokens]
in PSUM. A PE transpose (identity matmul) flips each 128-token slice back to
[128, 64] for the row-wise softmax/top-k chain.
"""

import os

import numpy as np

import concourse.bacc as bacc
import concourse.mybir as mybir
import concourse.tile as tile
from concourse import bass_utils
from concourse.masks import make_identity

N_TOKENS = 16384
D_MODEL = 4096
NUM_EXPERTS = 64
TOP_K = 2
EPS = 1e-8
N_CORES = 8

TPC = N_TOKENS // N_CORES  # tokens per core (2048)
P = 128                    # partitions
DC = D_MODEL // P          # contraction chunks (32)
TB = 512                   # token block (fp32 moving-free-dim / PSUM bank max)
NB = TPC // TB             # token blocks per core (4)
JT = TB // P               # 128-token tiles per block (4)

_PROGRAM = None
LAST_RESULT = None  # BassKernelResults of the most recent run (for test harness)


def _build_program():
    use_f32r = bool(int(os.environ.get("MOE_F32R", "0")))
    nc = bacc.Bacc(
        "TRN2",
        target_bir_lowering=False,
        debug=False,
        enable_asserts=False,
        num_devices=N_CORES,
    )
    f32 = mybir.dt.float32
    mm_dt = mybir.dt.float32r if use_f32r else f32
    xT = nc.dram_tensor("xT", [D_MODEL, TPC], f32, kind="ExternalInput")
    wS = nc.dram_tensor("wS", [P, DC * NUM_EXPERTS], f32, kind="ExternalInput")
    # Outputs stay in the kernel's [partition, block-tile, k] layout so the
    # store DMA is contiguous per partition; host reorders to token-major.
    scores = nc.dram_tensor("scores", [P, NB * JT, TOP_K], f32,
                            kind="ExternalOutput")
    idx = nc.dram_tensor("idx", [P, NB * JT, TOP_K], mybir.dt.uint32,
                         kind="ExternalOutput")
    pacc = nc.dram_tensor("pacc", [P, JT, NUM_EXPERTS], f32,
                          kind="ExternalOutput")

    E = NUM_EXPERTS
    with tile.TileContext(nc) as tc:
        with (
            tc.tile_pool(name="wp", bufs=1) as wpool,
            tc.tile_pool(name="xp", bufs=6) as xpool,
            tc.tile_pool(name="pp", bufs=2, space="PSUM") as pspool,
            tc.tile_pool(name="sp", bufs=3) as spool,
            tc.tile_pool(name="ap", bufs=1) as apool,
        ):
            # Gate weight, all 32 K-chunks resident: chunk d at wt_sb[:, d, :]
            wt_sb = wpool.tile([P, DC, E], f32)
            nc.sync.dma_start(wt_sb[:], wS.ap().rearrange("p (c e) -> p c e", c=DC))
            ident = wpool.tile([E, E], f32)
            make_identity(nc, ident[:])

            acc_sb = apool.tile([P, JT, E], f32)
            nc.vector.memset(acc_sb[:], 0.0)

            # Output staging: all 16 blocks' top-2 scores/indices, stored once
            sc_all = apool.tile([P, NB * JT, TOP_K], f32)
            ix_all = apool.tile([P, NB * JT, 8], mybir.dt.uint32)

            X = mybir.AxisListType.X
            for h in range(2):  # halves of this core's tokens (1024 each)
                # logits^T per 512-token sub-block, two half-sums stacked on
                # the partition axis: [0:64] = even d-chunks, [64:128] = odd.
                # The two col-groups of the PE array run concurrently
                # (tile_position), using all 128 columns despite M=64.
                pT = [
                    pspool.tile([P, TB], f32, tag="pT", name=f"pT{h}_{b}", bufs=4)
                    for b in range(2)
                ]
                for dp in range(DC // 2):
                    xt = xpool.tile([P, 2, 2 * TB], f32, tag="xt",
                                    name=f"xt{h}_{dp}", bufs=14)
                    src_ap = (
                        xT.ap()[2 * dp * P:(2 * dp + 2) * P,
                                h * 2 * TB:(h + 1) * 2 * TB]
                        .rearrange("(c p) n -> p c n", c=2)
                    )
                    if h == 0 and dp == 0:
                        # First tile: quarter-loads so matmul 0 starts early.
                        for c in range(2):
                            for b in range(2):
                                eng = nc.sync if (c, b) in ((0, 0), (1, 1)) else nc.scalar
                                eng.dma_start(
                                    xt[:, c, b * TB:(b + 1) * TB],
                                    src_ap[:, c, b * TB:(b + 1) * TB],
                                )
                    else:
                        dma_eng = nc.sync if dp % 2 == 0 else nc.scalar
                        dma_eng.dma_start(xt[:], src_ap)
                    for b in range(2):
                        for c in range(2):
                            nc.tensor.matmul(
                                pT[b][c * E:(c + 1) * E, :],
                                wt_sb[:, 2 * dp + c, :].bitcast(mm_dt),
                                xt[:, c, b * TB:(b + 1) * TB].bitcast(mm_dt),
                                start=(dp == 0),
                                stop=(dp == DC // 2 - 1),
                                tile_position=(0, c * E),
                                skip_group_check=True,
                            )
                for b in range(2):
                    tb = 2 * h + b
                    lt = spool.tile([E, TB], f32, tag="lt")
                    nc.vector.tensor_copy(lt[:], pT[b][0:E, :])
                    nc.vector.tensor_add(lt[:], lt[:], pT[b][E:2 * E, :])
                    # [128 tokens, 4 tiles, 64 experts] logits, one PSUM bank
                    p4 = pspool.tile([P, JT, E], f32, tag="p4",
                                     name=f"p4_{tb}", bufs=2)
                    for j in range(JT):
                        nc.tensor.transpose(
                            p4[:, j, :], lt[:, j * P:(j + 1) * P], ident[:]
                        )
                    ex = spool.tile([P, JT, E], f32, tag="ex")
                    nc.scalar.activation(
                        ex[:], p4[:], mybir.ActivationFunctionType.Exp
                    )
                    s = spool.tile([P, JT], f32, tag="s")
                    nc.vector.reduce_sum(s[:], ex[:], axis=X)
                    r = spool.tile([P, JT], f32, tag="r")
                    nc.vector.reciprocal(r[:], s[:])
                    probs = spool.tile([P, JT, E], f32, tag="probs")
                    nc.vector.tensor_mul(
                        probs[:], ex[:], r[:].to_broadcast([P, JT, E])
                    )
                    nc.vector.tensor_add(acc_sb[:], acc_sb[:], probs[:])

                    # top-2 on exp values (same order as probs; norm cancels)
                    max84 = spool.tile([P, JT, 8], f32, tag="max84")
                    for j in range(JT):
                        nc.vector.max(out=max84[:, j, :], in_=ex[:, j, :])
                    for j in range(JT):
                        nc.vector.max_index(
                            ix_all[:, tb * JT + j, :], max84[:, j, :], ex[:, j, :]
                        )
                    s2 = spool.tile([P, JT], f32, tag="s2")
                    nc.vector.reduce_sum(s2[:], max84[:, :, 0:TOP_K], axis=X)
                    r2 = spool.tile([P, JT], f32, tag="r2")
                    nc.vector.reciprocal(r2[:], s2[:])
                    nc.vector.tensor_mul(
                        sc_all[:, tb * JT:(tb + 1) * JT, :],
                        max84[:, :, 0:TOP_K],
                        r2[:].to_broadcast([P, JT, TOP_K])
                    )

            nc.sync.dma_start(scores.ap(), sc_all[:])
            nc.scalar.dma_start(idx.ap(), ix_all[:, :, 0:TOP_K])
            nc.gpsimd.dma_start(pacc.ap()[:], acc_sb[:])

    nc.compile()
    return nc


def get_program():
    global _PROGRAM
    if _PROGRAM is None:
        _PROGRAM = _build_program()
    return _PROGRAM


def kernel(x: np.ndarray, W: np.ndarray):
    global LAST_RESULT
    if not os.environ.get("MOE_TRACE"):
        # Profiling needs an NTFF hook this environment may not provide;
        # never let a stray BASS_TRACE turn it on during grading.
        os.environ["BASS_NEVER_TRACE"] = "1"
    nc = get_program()
    x = np.asarray(x, dtype=np.float32)
    W = np.asarray(W, dtype=np.float32)
    # wS[p, c*E+e] = W[e, c*128+p] — contiguous per-partition weight load
    wS = np.ascontiguousarray(
        W.T.reshape(DC, P, NUM_EXPERTS).transpose(1, 0, 2).reshape(P, DC * NUM_EXPERTS)
    )
    in_maps = [
        {
            "xT": np.ascontiguousarray(x[i * TPC:(i + 1) * TPC, :].T),
            "wS": wS,
        }
        for i in range(N_CORES)
    ]
    res = bass_utils.run_bass_kernel_spmd(nc, in_maps, core_ids=list(range(N_CORES)))
    LAST_RESULT = res
    outs = res.results

    # device layout [p, g, k] -> token-major (t = g*128 + p)
    topk_scores = np.concatenate(
        [np.asarray(o["scores"]).transpose(1, 0, 2).reshape(TPC, TOP_K)
         for o in outs], axis=0
    )
    topk_indices = np.concatenate(
        [np.asarray(o["idx"]).transpose(1, 0, 2).reshape(TPC, TOP_K)
         for o in outs], axis=0
    ).astype(np.uint32).view(np.int32)
    part = np.stack([np.asarray(o["pacc"]) for o in outs], axis=0)
    expert_prob = part.astype(np.float64).sum(axis=(0, 1, 2)) / N_TOKENS
    loss = np.float32(np.sum(expert_prob * np.log(expert_prob + EPS)))
    return topk_scores, topk_indices, loss


if __name__ == "__main__":
    rng = np.random.default_rng(0)
    x = rng.standard_normal((N_TOKENS, D_MODEL), dtype=np.float32)
    W = rng.standard_normal((NUM_EXPERTS, D_MODEL), dtype=np.float32) / np.sqrt(D_MODEL)
    s, i, l = kernel(x, W)
    print(s.shape, s.dtype, i.shape, i.dtype, l)


# revision 22
# speedup vs baseline: 1.1060x; 1.1060x over previous
"""MoE gate (softmax + top-2 + load-balance loss) on 8 Trainium2 NeuronCores.

Data-parallel: token dim (16384) sharded 8 ways, gate weight replicated.
Per core: logits = x_shard @ W.T via PE (f32), softmax + top-2 via DVE/ACT,
partial per-expert prob sums returned per core; the scalar load-balance loss
is finished on host (tiny reduction).

x arrives [n, d] but the PE contracts along the partition axis, so each
core's shard is fed pre-transposed ([d, n], done on host) — natural-layout
DMA loads, no on-device transpose.

Matmul arrangement: the small gate weight W^T chunk [128, 64] is the
stationary operand (cheap reload), x streams as the moving operand with the
full 512-wide free dim, producing logits transposed [64 experts, 512 tokens]
in PSUM. A PE transpose (identity matmul) flips each 128-token slice back to
[128, 64] for the row-wise softmax/top-k chain.
"""

import os

import numpy as np

import concourse.bacc as bacc
import concourse.mybir as mybir
import concourse.tile as tile
from concourse import bass_utils
from concourse.masks import make_identity

N_TOKENS = 16384
D_MODEL = 4096
NUM_EXPERTS = 64
TOP_K = 2
EPS = 1e-8
N_CORES = 8

TPC = N_TOKENS // N_CORES  # tokens per core (2048)
P = 128                    # partitions
DC = D_MODEL // P          # contraction chunks (32)
TB = 512                   # token block (fp32 moving-free-dim / PSUM bank max)
NB = TPC // TB             # token blocks per core (4)
JT = TB // P               # 128-token tiles per block (4)

_PROGRAM = None
LAST_RESULT = None  # BassKernelResults of the most recent run (for test harness)


def _build_program():
    use_f32r = bool(int(os.environ.get("MOE_F32R", "0")))
    nc = bacc.Bacc(
        "TRN2",
        target_bir_lowering=False,
        debug=False,
        enable_asserts=False,
        num_devices=N_CORES,
    )
    f32 = mybir.dt.float32
    mm_dt = mybir.dt.float32r if use_f32r else f32
    xT = nc.dram_tensor("xT", [D_MODEL, TPC], f32, kind="ExternalInput")
    wS = nc.dram_tensor("wS", [P, DC * NUM_EXPERTS], f32, kind="ExternalInput")
    # Outputs stay in the kernel's [partition, block-tile, k] layout so the
    # store DMA is contiguous per partition; host reorders to token-major.
    scores = nc.dram_tensor("scores", [P, NB * JT, TOP_K], f32,
                            kind="ExternalOutput")
    idx = nc.dram_tensor("idx", [P, NB * JT, TOP_K], mybir.dt.uint32,
                         kind="ExternalOutput")
    pacc = nc.dram_tensor("pacc", [P, JT, NUM_EXPERTS], f32,
                          kind="ExternalOutput")

    E = NUM_EXPERTS
    with tile.TileContext(nc) as tc:
        with (
            tc.tile_pool(name="wp", bufs=1) as wpool,
            tc.tile_pool(name="xp", bufs=6) as xpool,
            tc.tile_pool(name="pp", bufs=2, space="PSUM") as pspool,
            tc.tile_pool(name="sp", bufs=3) as spool,
            tc.tile_pool(name="ap", bufs=1) as apool,
        ):
            # Gate weight, all 32 K-chunks resident: chunk d at wt_sb[:, d, :]
            wt_sb = wpool.tile([P, DC, E], f32)
            nc.sync.dma_start(wt_sb[:], wS.ap().rearrange("p (c e) -> p c e", c=DC))
            ident = wpool.tile([E, E], f32)
            make_identity(nc, ident[:])

            acc_sb = apool.tile([P, JT, E], f32)
            nc.vector.memset(acc_sb[:], 0.0)

            # Output staging: all 16 blocks' top-2 scores/indices, stored once
            sc_all = apool.tile([P, NB * JT, TOP_K], f32)
            ix_all = apool.tile([P, NB * JT, 8], mybir.dt.uint32)

            X = mybir.AxisListType.X
            for h in range(2):  # halves of this core's tokens (1024 each)
                # logits^T per 512-token sub-block, two half-sums stacked on
                # the partition axis: [0:64] = even d-chunks, [64:128] = odd.
                # The two col-groups of the PE array run concurrently
                # (tile_position), using all 128 columns despite M=64.
                pT = [
                    pspool.tile([P, TB], f32, tag="pT", name=f"pT{h}_{b}", bufs=4)
                    for b in range(2)
                ]
                for dp in range(DC // 2):
                    xt = xpool.tile([P, 2, 2 * TB], f32, tag="xt",
                                    name=f"xt{h}_{dp}", bufs=14)
                    src_ap = (
                        xT.ap()[2 * dp * P:(2 * dp + 2) * P,
                                h * 2 * TB:(h + 1) * 2 * TB]
                        .rearrange("(c p) n -> p c n", c=2)
                    )
                    if h == 0 and dp == 0:
                        # First tile: quarter-loads so matmul 0 starts early.
                        for c in range(2):
                            for b in range(2):
                                eng = nc.sync if (c, b) in ((0, 0), (1, 1)) else nc.scalar
                                eng.dma_start(
                                    xt[:, c, b * TB:(b + 1) * TB],
                                    src_ap[:, c, b * TB:(b + 1) * TB],
                                )
                    else:
                        dma_eng = nc.sync if dp % 2 == 0 else nc.scalar
                        dma_eng.dma_start(xt[:], src_ap)
                    for b in range(2):
                        for c in range(2):
                            nc.tensor.matmul(
                                pT[b][c * E:(c + 1) * E, :],
                                wt_sb[:, 2 * dp + c, :].bitcast(mm_dt),
                                xt[:, c, b * TB:(b + 1) * TB].bitcast(mm_dt),
                                start=(dp == 0),
                                stop=(dp == DC // 2 - 1),
                                tile_position=(0, c * E),
                                skip_group_check=True,
                            )
                for b in range(2):
                    tb = 2 * h + b
                    lt = spool.tile([E, TB], f32, tag="lt")
                    nc.vector.tensor_copy(lt[:], pT[b][0:E, :])
                    nc.vector.tensor_add(lt[:], lt[:], pT[b][E:2 * E, :])
                    # [128 tokens, 4 tiles, 64 experts] logits, one PSUM bank
                    p4 = pspool.tile([P, JT, E], f32, tag="p4",
                                     name=f"p4_{tb}", bufs=2)
                    for j in range(JT):
                        nc.tensor.transpose(
                            p4[:, j, :], lt[:, j * P:(j + 1) * P], ident[:]
                        )
                    ex = spool.tile([P, JT, E], f32, tag="ex")
                    nc.scalar.activation(
                        ex[:], p4[:], mybir.ActivationFunctionType.Exp
                    )
                    s = spool.tile([P, JT], f32, tag="s")
                    nc.vector.reduce_sum(s[:], ex[:], axis=X)
                    r = spool.tile([P, JT], f32, tag="r")
                    nc.vector.reciprocal(r[:], s[:])
                    probs = spool.tile([P, JT, E], f32, tag="probs")
                    nc.gpsimd.tensor_mul(
                        probs[:], ex[:], r[:].to_broadcast([P, JT, E])
                    )
                    nc.gpsimd.tensor_add(acc_sb[:], acc_sb[:], probs[:])

                    # top-2 on exp values (same order as probs; norm cancels)
                    max84 = spool.tile([P, JT, 8], f32, tag="max84")
                    for j in range(JT):
                        nc.vector.max(out=max84[:, j, :], in_=ex[:, j, :])
                    for j in range(JT):
                        nc.vector.max_index(
                            ix_all[:, tb * JT + j, :], max84[:, j, :], ex[:, j, :]
                        )
                    s2 = spool.tile([P, JT], f32, tag="s2")
                    nc.vector.reduce_sum(s2[:], max84[:, :, 0:TOP_K], axis=X)
                    r2 = spool.tile([P, JT], f32, tag="r2")
                    nc.vector.reciprocal(r2[:], s2[:])
                    nc.vector.tensor_mul(
                        sc_all[:, tb * JT:(tb + 1) * JT, :],
                        max84[:, :, 0:TOP_K],
                        r2[:].to_broadcast([P, JT, TOP_K])
                    )

            nc.sync.dma_start(scores.ap(), sc_all[:])
            nc.scalar.dma_start(idx.ap(), ix_all[:, :, 0:TOP_K])
            nc.gpsimd.dma_start(pacc.ap()[:], acc_sb[:])

    nc.compile()
    return nc


def get_program():
    global _PROGRAM
    if _PROGRAM is None:
        _PROGRAM = _build_program()
    return _PROGRAM


def kernel(x: np.ndarray, W: np.ndarray):
    global LAST_RESULT
    if not os.environ.get("MOE_TRACE"):
        # Profiling needs an NTFF hook this environment may not provide;
        # never let a stray BASS_TRACE turn it on during grading.
        os.environ["BASS_NEVER_TRACE"] = "1"
    nc = get_program()
    x = np.asarray(x, dtype=np.float32)
    W = np.asarray(W, dtype=np.float32)
    # wS[p, c*E+e] = W[e, c*128+p] — contiguous per-partition weight load
    wS = np.ascontiguousarray(
        W.T.reshape(DC, P, NUM_EXPERTS).transpose(1, 0, 2).reshape(P, DC * NUM_EXPERTS)
    )
    in_maps = [
        {
            "xT": np.ascontiguousarray(x[i * TPC:(i + 1) * TPC, :].T),
            "wS": wS,
        }
        for i in range(N_CORES)
    ]
    res = bass_utils.run_bass_kernel_spmd(nc, in_maps, core_ids=list(range(N_CORES)))
    LAST_RESULT = res
    outs = res.results

    # device layout [p, g, k] -> token-major (t = g*128 + p)
    topk_scores = np.concatenate(
        [np.asarray(o["scores"]).transpose(1, 0, 2).reshape(TPC, TOP_K)
         for o in outs], axis=0
    )
    topk_indices = np.concatenate(
        [np.asarray(o["idx"]).transpose(1, 0, 2).reshape(TPC, TOP_K)
         for o in outs], axis=0
    ).astype(np.uint32).view(np.int32)
    part = np.stack([np.asarray(o["pacc"]) for o in outs], axis=0)
    expert_prob = part.astype(np.float64).sum(axis=(0, 1, 2)) / N_TOKENS
    loss = np.float32(np.sum(expert_prob * np.log(expert_prob + EPS)))
    return topk_scores, topk_indices, loss


if __name__ == "__main__":
    rng = np.random.default_rng(0)
    x = rng.standard_normal((N_TOKENS, D_MODEL), dtype=np.float32)
    W = rng.standard_normal((NUM_EXPERTS, D_MODEL), dtype=np.float32) / np.sqrt(D_MODEL)
    s, i, l = kernel(x, W)
    print(s.shape, s.dtype, i.shape, i.dtype, l)
